# revision 10
# baseline (speedup 1.0000x reference)
"""Bidirectional quantized RNN (fake-quant int8 weights/acts) on 8 trn2 cores.

Sequence-parallel sharding as before: each direction split into 16 chunks of
L=seq/16 steps with W warmup steps (outputs discarded, chunk 0 exact); core c
handles direction c//4 and chunks 4*(c%4)..4*(c%4)+3 -> 64 columns, S=L+W
steps.  The recurrent state is the bf16 tanh output th (validated: bf16
rounding is washed out by the contraction).

This version restructures the per-step work around the TimelineSim cost
model (matmul cost = out_free_rows * pe_cycle * cyc_per_row; fp8e4 DoubleRow
= 0.5 cyc/row; ACT op = free*0.833ns + ~185ns fixed):

* 2 pipes x 32 columns (not 4x16): halves the ACT fixed-overhead per step
  (2 ops -> 583ns/step) while keeping enough chain parallelism.
* x-side matmuls run as EXACT fp8e4 DoubleRow at 0.75x the bf16 row cost:
  j = j8 + dj with j8 = e4m3(j) and dj = j - j8 (always a small int <= 4,
  e4m3-exact); k_ri = 16*kh + kl with kh,kl in [-8,8] (e4m3-exact).
    DR1(ic):  j8 @ 16kh_ic  +  dj @ 16kh_ic   (= j @ 16kh_ic, exact)
    DR2(icA,icB): j8_A @ kl_A + j8_B @ kl_B   (pairs two ic blocks)
  dropping only dj@kl (~0.002% of gate RMS).  12 row-equivs/col-step vs 16.
* bias seeded by one fp8 DR matmul per (nck, window): bias_int decomposed
  into 4 base-16 digits (rows 16*A,16*B,16*C,16*D vs const rows
  128,8,0.5,1/32) -> 2 row-equivs/col-step vs 4.
* recurrent matmuls stay EXACT bf16 (th moving, bf16(127*k_rh) stationary).
* one consolidated x DMA per window and one output DMA per slab.

Per step: 32 m-matmuls (427ns) + 7 window fillers (373ns) = 800ns PE, ACT
583ns hidden, chain round-trip ~830ns < 2*800 -> ~112us vs 146.5us baseline.
"""
import numpy as np
import ml_dtypes
from contextlib import ExitStack

import concourse.bacc as bacc
import concourse.tile as tile
import concourse.mybir as mybir
from concourse.bass_utils import run_bass_kernel_spmd

SEQ, BATCH, IN, HID = 2048, 16, 512, 512
QMAX = np.float32(127.0)
F32 = mybir.dt.float32
BF16 = mybir.dt.bfloat16
FP8 = mybir.dt.float8e4
ACTF = mybir.ActivationFunctionType
DR = mybir.MatmulPerfMode.DoubleRow
E4 = ml_dtypes.float8_e4m3

NCHUNK = 16     # chunks per direction
WARM = 8        # cold-start warmup steps per chunk (chunk 0: exact anyway)
XB = 8          # steps per PSUM gate window / x block
NCOL = (NCHUNK // 4) * BATCH  # columns per core = 64
NP = 2          # pipes
PC = NCOL // NP               # columns per pipe = 32

_cache = {}


def _cache_key(seq):
    return (seq, 128 if seq >= 128 else 32)


def _slab_schedule(S):
    if S % 8 == 0 and S >= 16:
        return [8] * (S // 8 - 1) + [4, 4]
    for ob in (8, 17, 16, 34):
        if S % ob == 0:
            return [ob] * (S // ob)
    raise ValueError(S)


def _build(S, N):
    """One SPMD program for all 8 cores. S = L+W local steps, N=64 columns."""
    assert N == NCOL
    slabs = _slab_schedule(S)
    slab_start = []
    t0 = 0
    for ln in slabs:
        slab_start.append(t0)
        t0 += ln
    slab_of = {}
    for i, (st, ln) in enumerate(zip(slab_start, slabs)):
        for t in range(st, st + ln):
            slab_of[t] = (i, st, ln)
    nc = bacc.Bacc("TRN2")
    nxb = S // XB
    # x packed [128, pipe, ic, window, 2*256]: per (p,ic,window) the 512-byte
    # block is [kind(j8,dj), t_local(8), col(32)] contiguous -> 512B descs.
    x_p = nc.declare_dram_parameter("x", [128, NP, 4, nxb, 512], FP8,
                                    isOutput=False)
    # DR1 stationary: 16*kh duplicated on both k-tiles
    wkh_p = nc.declare_dram_parameter("wkh", [128, 4, 2, HID], FP8,
                                      isOutput=False)
    # DR2 stationary: kl, paired across ic via stride-2 slicing
    wkl_p = nc.declare_dram_parameter("wkl", [128, 4, HID], FP8,
                                      isOutput=False)
    # recurrent weights bf16(127*k_rh), kc-major blocks on partitions
    wrh_p = nc.declare_dram_parameter("wrh", [128, 4, HID], BF16,
                                      isOutput=False)
    # bias digit rows [128, 2, nck, 128]: rows 0..3 of tile0 = 16*digit
    bw_p = nc.declare_dram_parameter("bw", [128, 2, 4, 128], FP8,
                                     isOutput=False)
    # bias moving consts: rows 0..3 of tile0 = 128, 8, 0.5, 1/32
    bx_p = nc.declare_dram_parameter("bx", [128, 2, 256], FP8, isOutput=False)
    cf_p = nc.declare_dram_parameter("cf", [128, 1], F32, isOutput=False)
    # th output, all pipes merged: [p, t, nck, col]
    out_p = nc.declare_dram_parameter("out", [128, S, 4, N], BF16,
                                      isOutput=True)

    with tile.TileContext(nc) as tc, ExitStack() as ctx:
        const = ctx.enter_context(tc.tile_pool(name="const", bufs=1))
        # load order: bias weights first (bias matmuls seed window 0), then
        # x-weights, then recurrent weights, scale last.
        bw_sb = const.tile([128, 2, 4, 128], FP8, tag="bw")
        nc.sync.dma_start(bw_sb[:], bw_p[:])
        bx_sb = const.tile([128, 2, 256], FP8, tag="bx")
        nc.sync.dma_start(bx_sb[:], bx_p[:])
        cf_sb = const.tile([128, 1], F32, tag="cf")
        nc.sync.dma_start(cf_sb[:], cf_p[:])
        wkh_sb = const.tile([128, 4, 2, HID], FP8, tag="wkh")
        nc.gpsimd.dma_start(wkh_sb[:], wkh_p[:])
        wkl_sb = const.tile([128, 4, HID], FP8, tag="wkl")
        nc.gpsimd.dma_start(wkl_sb[:], wkl_p[:])
        wrh_t = const.tile([128, 4, HID], BF16, tag="wrh")
        nc.gpsimd.dma_start(wrh_t[:], wrh_p[:])
        wrh_sb = [wrh_t[:, kc, :] for kc in range(4)]
        # Warm the ACT tanh table early.
        warm = const.tile([128, 1], F32, tag="warm")
        nc.scalar.activation(warm[:, 0:1], cf_sb[:, 0:1], ACTF.Tanh)

        pJ = ctx.enter_context(tc.tile_pool(name="pJ", bufs=5))
        pM = ctx.enter_context(tc.tile_pool(name="pM", bufs=3))
        psG = [ctx.enter_context(tc.tile_pool(name=f"psG{p}", bufs=2,
                                              space="PSUM")) for p in range(NP)]

        j_tiles = [None] * nxb
        g_tiles = [[None] * nxb for _ in range(NP)]

        def dma_x(b):
            jt = pJ.tile([128, NP, 4, 2, 256], FP8, name="j", tag="j")
            nc.sync.dma_start(jt[:], x_p[:, :, :, b, :])
            j_tiles[b] = jt

        jmm_queue = []  # deferred window-seeding matmuls, drained as PE filler

        def push_window(b):
            jt = j_tiles[b]
            for p in range(NP):
                g = psG[p].tile([128, 4, 256], F32, name="g", tag="g")
                g_tiles[p][b] = g
                # bias DR seeds; nck0/nck2 lead their 2KB PSUM banks (start
                # marks the whole bank pending-zero; later writes to pending
                # bytes overwrite).
                for nck in (0, 2, 1, 3):
                    jmm_queue.append((
                        g[:, nck, :], bw_sb[:, :, nck, :], bx_sb[:],
                        nck % 2 == 0, DR))
                # DR1: (j8, dj) x 16kh_ic  == j @ 16kh_ic exactly
                for ic in range(4):
                    for nck in range(4):
                        jmm_queue.append((
                            g[:, nck, :],
                            wkh_sb[:, ic, :, nck * 128:(nck + 1) * 128],
                            jt[:, p, ic, :, :],
                            False, DR))
                # DR2: (j8_icA @ kl_icA) + (j8_icB @ kl_icB), pairs (0,2),(1,3)
                for ica in range(2):
                    for nck in range(4):
                        jmm_queue.append((
                            g[:, nck, :],
                            wkl_sb[:, ica:ica + 3:2, nck * 128:(nck + 1) * 128],
                            jt[:, p, ica:ica + 3:2, 0, :],
                            False, DR))

        def emit_jmm(n):
            for _ in range(n):
                if not jmm_queue:
                    return
                out, lhsT, rhs, start, pm = jmm_queue.pop(0)
                nc.tensor.matmul(out, lhsT, rhs, start=start, stop=False,
                                 perf_mode=pm, skip_group_check=True)

        dma_x(0)
        dma_x(1)
        dma_x(2)
        push_window(0)
        emit_jmm(len(jmm_queue))
        push_window(1)

        m0 = pM.tile([128, 1, 4, N], BF16, name="m0", tag="m0")
        nc.vector.memset(m0[:], 0.0)
        m_prev = [m0] * NP
        prev_slot = [0] * NP
        mslab = None

        for t in range(S):
            b, s = t // XB, t % XB
            sb_i, sb_st, sb_ln = slab_of[t]
            os = t - sb_st
            if s == 0 and 3 <= b + 3 < nxb:
                dma_x(b + 3)
            if os == 0:
                mslab = pM.tile([128, sb_ln, 4, N], BF16, name="m", tag="m")
            for p in range(NP):
                gate = g_tiles[p][b]
                c0 = p * PC
                for kc in range(4):
                    for nck in range(4):
                        nc.tensor.matmul(
                            gate[:, nck, s * PC:(s + 1) * PC],
                            wrh_sb[kc][:, nck * 128:(nck + 1) * 128],
                            m_prev[p][:, prev_slot[p], kc, c0:c0 + PC],
                            start=False, stop=(kc == 3 and nck == 3),
                            skip_group_check=True,
                        )
                emit_jmm(3 if p == 0 else 4)
                nc.scalar.activation(mslab[:, os, :, c0:c0 + PC],
                                     gate[:, :, s * PC:(s + 1) * PC],
                                     ACTF.Tanh, scale=cf_sb[:, 0:1])
                m_prev[p], prev_slot[p] = mslab, os
            if s == XB - 1 and b + 2 < nxb:
                push_window(b + 2)
            if os == sb_ln - 1:
                eng = nc.scalar if sb_i == len(slabs) - 1 else nc.sync
                eng.dma_start(out_p[:, sb_st:sb_st + sb_ln, :, :], mslab[:])
    nc.compile()
    return nc


def _host_prep(inputs, seq):
    L = seq // NCHUNK
    S = L + WARM
    x = np.asarray(inputs["inputs"], np.float32)
    in_maps = []
    meta = []
    for d, (wri, wrh, b) in enumerate([
        (inputs["w_ri_f"], inputs["w_rh_f"], inputs["b_f"]),
        (inputs["w_ri_b"], inputs["w_rh_b"], inputs["b_b"]),
    ]):
        wri = np.asarray(wri, np.float32); wrh = np.asarray(wrh, np.float32)
        b = np.asarray(b, np.float32)
        threshold = np.float32(max(np.abs(wri).max(), np.abs(wrh).max()))
        s = np.float32(threshold / QMAX)
        k_ri = np.clip(np.round(wri / s), -QMAX, QMAX)
        k_rh = np.clip(np.round(wrh / s), -QMAX, QMAX)
        c_s = np.float32(np.float64(s) / 127.0)
        # x-weight split: k_ri = 16*kh + kl, kh/kl in [-8,8] (e4m3-exact)
        kh = np.round(k_ri / 16.0)
        kl = k_ri - 16.0 * kh
        assert np.abs(kh).max() <= 8 and np.abs(kl).max() <= 8
        # [128, ic, HID] layouts (partition = channel within ic block)
        wkh = (16.0 * kh).reshape(4, 128, HID).transpose(1, 0, 2)
        wkhd = np.repeat(wkh[:, :, None, :], 2, axis=2)  # dup on k-tiles
        wkl2 = kl.reshape(4, 128, HID).transpose(1, 0, 2)
        wrh_p = (127.0 * k_rh).reshape(4, 128, HID).transpose(1, 0, 2)
        # bias digits: bias = 2048A + 128B + 8C + D/2, digits in [-8,8]
        bias_int = b.astype(np.float64) / np.float64(c_s)
        A = np.round(bias_int / 2048.0); r = bias_int - 2048.0 * A
        B = np.round(r / 128.0); r -= 128.0 * B
        C = np.round(r / 8.0); r -= 8.0 * C
        D = np.round(2.0 * r)
        assert max(np.abs(A).max(), np.abs(B).max(), np.abs(C).max(),
                   np.abs(D).max()) <= 8
        bw = np.zeros((128, 2, 4, 128), np.float64)
        for r_i, dig in enumerate((A, B, C, D)):
            bw[r_i, 0] = (16.0 * dig).reshape(4, 128)
        cf = np.full((128, 1), c_s, np.float32)
        bx = np.zeros((128, 2, 256), np.float32)
        for r_i, v in enumerate((128.0, 8.0, 0.5, 0.03125)):
            bx[r_i, 0, :] = v
        meta.append((np.ascontiguousarray(wkhd.astype(E4)),
                     np.ascontiguousarray(wkl2.astype(E4)),
                     np.ascontiguousarray(wrh_p.astype(ml_dtypes.bfloat16)),
                     np.ascontiguousarray(bw.astype(E4)),
                     np.ascontiguousarray(bx.astype(E4)), cf))
    xs = [x[:seq], x[:seq][::-1]]
    nxb = S // XB
    for core in range(8):
        d = core // 4
        wkhd, wkl2, wrh_p, bw, bx, cf = meta[d]
        xd = xs[d]
        xT = np.empty((128, 4, S, NCOL), np.float32)
        for cl in range(NCHUNK // 4):
            q = 4 * (core % 4) + cl
            t0 = 0 if q == 0 else q * L - WARM
            blk = xd[t0:t0 + S]                     # [S, 16, 512]
            xT[:, :, :, cl * 16:(cl + 1) * 16] = (
                blk.transpose(2, 0, 1).reshape(4, 128, S, 16).transpose(1, 0, 2, 3))
        j = np.clip(np.round(127.0 * np.clip(xT, -1.0, 1.0)), -127.0, 127.0)
        j8 = j.astype(E4)
        dj = j - j8.astype(np.float32)
        assert np.abs(dj).max() <= 4
        # pack [128, pipe, ic, window, kind*256 + t*32 + c]
        def pack(v):  # v [128, 4ic, S, 64]
            v = v.reshape(128, 4, nxb, XB, NP, PC)
            return v.transpose(0, 4, 1, 2, 3, 5)  # [128, NP, 4, nxb, 8, 32]
        xp = np.stack([pack(j8.astype(np.float32)), pack(dj)], axis=4)
        xp = np.ascontiguousarray(
            xp.reshape(128, NP, 4, nxb, 512).astype(E4))
        in_maps.append({"x": xp, "wkh": wkhd, "wkl": wkl2, "wrh": wrh_p,
                        "bw": bw, "bx": bx, "cf": cf})
    return in_maps


def _run(inputs, seq=SEQ, tb=None, trace=False):
    L = seq // NCHUNK
    S = L + WARM
    assert seq % NCHUNK == 0 and S % XB == 0
    key = _cache_key(seq)
    if key not in _cache:
        _cache[key] = _build(S, NCOL)
    nc = _cache[key]
    in_maps = _host_prep(inputs, seq)
    res = run_bass_kernel_spmd(nc, in_maps, core_ids=list(range(8)), trace=trace)
    out = np.empty((seq, BATCH, 2 * HID), np.float32)
    for core in range(8):
        d = core // 4
        th = np.asarray(res.results[core]["out"], dtype=np.float32)
        m = np.clip(np.round(127.0 * th), -127.0, 127.0)
        h = m / np.float32(127.0)
        h = h.transpose(1, 3, 2, 0).reshape(S, NCOL, HID)  # [S, n, hid]
        for cl in range(NCHUNK // 4):
            q = 4 * (core % 4) + cl
            lo = 0 if q == 0 else WARM
            sl = h[lo:lo + L, cl * 16:(cl + 1) * 16, :]    # [L, 16, 512]
            if d == 0:
                out[q * L:(q + 1) * L, :, :HID] = sl
            else:
                out[seq - (q + 1) * L:seq - q * L, :, HID:] = sl[::-1]
    return out, res


def kernel(**inputs):
    out, _ = _run(inputs)
    return out


# revision 11
# speedup vs baseline: 1.1732x; 1.1732x over previous
"""Bidirectional quantized RNN (fake-quant int8 weights/acts) on 8 trn2 cores.

Sequence-parallel sharding: each direction split into NCHUNK chunks of
L=seq/NCHUNK steps with W warmup steps (outputs discarded, chunk 0 exact);
core c handles direction c//4 and chunks (NCHUNK//4)*(c%4).. -> NCOL columns,
S=L+W steps.  The recurrent state is the bf16 tanh output th (validated:
bf16 rounding is washed out by the contraction).

Geometry tuned against the TimelineSim cost model (matmul cost =
out_free_rows * pe_cycle * cyc_per_row with fp8e4 DoubleRow = 0.5 cyc/row;
ACT op = free*0.833ns + ~185ns access + ~450ns round-trip latency to the
next dependent matmul):

* NCHUNK=32 -> 128 columns/core over 4 pipes x 32 cols, S=72 steps: the
  tanh->matmul round trip (~950ns) hides under ~1600ns/step of PE work,
  and each ACT op is big enough (free=128) to amortize its fixed cost.
* XB=4-step PSUM gate windows: 4 pipes x 2 windows x 1 bank = 8 banks.
* x-side matmuls in fp8e4 DoubleRow, EXACT at 0.75x bf16 row cost:
  j = j8 + dj (j8 = e4m3(j); dj = j - j8 is a small int <= 4, e4m3-exact);
  k_ri = 16*kh + kl (kh, kl in [-8,8], e4m3-exact).
    DR1(ic):      j8 @ 16kh_ic + dj @ 16kh_ic  (= j @ 16kh_ic exact)
    DR2(icA,icB): j8_A @ kl_A  + j8_B @ kl_B   (pairs two ic blocks)
  dropping only dj@kl (~0.002% of gate RMS).  J_MODE='round8' instead packs
  (j8@16kh + j8@kl) in ONE DR per ic (0.5x cost, ~1.5% gate noise).
* bias seeded by one fp8 DR matmul per (nck, window): bias_int = 2048A +
  128B + 8C + D/2 digit rows (e4m3-exact) against const rows 128,8,.5,1/32.
* recurrent matmuls stay EXACT bf16 (th moving, bf16(127*k_rh) stationary).
* x DMAs fetch two windows at a time (512B descriptors); one output DMA
  per 8-step slab.
"""
import numpy as np
import ml_dtypes
from contextlib import ExitStack

import concourse.bacc as bacc
import concourse.tile as tile
import concourse.mybir as mybir
from concourse.bass_utils import run_bass_kernel_spmd

SEQ, BATCH, IN, HID = 2048, 16, 512, 512
QMAX = np.float32(127.0)
F32 = mybir.dt.float32
BF16 = mybir.dt.bfloat16
FP8 = mybir.dt.float8e4
ACTF = mybir.ActivationFunctionType
DR = mybir.MatmulPerfMode.DoubleRow
E4 = ml_dtypes.float8_e4m3

NCHUNK = 32     # chunks per direction
WARM = 8        # cold-start warmup steps per chunk (chunk 0: exact anyway)
XB = 4          # steps per PSUM gate window / x block
NCOL = (NCHUNK // 4) * BATCH  # columns per core = 128
NP = 4          # pipes
PC = NCOL // NP               # columns per pipe = 32
J_MODE = "exact12"            # 'exact12' | 'round8'

_cache = {}


def _cache_key(seq):
    return (seq, 128 if seq >= 128 else 32, J_MODE, WARM, NCHUNK)


def _slab_schedule(S):
    if S % 8 == 0 and S >= 16:
        return [8] * (S // 8 - 1) + [4, 4]
    for ob in (8, 17, 16, 34):
        if S % ob == 0:
            return [ob] * (S // ob)
    raise ValueError(S)


def _build(S, N):
    """One SPMD program for all 8 cores. S = L+W local steps, N columns."""
    assert N == NCOL
    slabs = _slab_schedule(S)
    slab_start = []
    t0 = 0
    for ln in slabs:
        slab_start.append(t0)
        t0 += ln
    slab_of = {}
    for i, (st, ln) in enumerate(zip(slab_start, slabs)):
        for t in range(st, st + ln):
            slab_of[t] = (i, st, ln)
    nc = bacc.Bacc("TRN2")
    nxb = S // XB
    npair = nxb // 2
    MV = XB * PC  # moving size per DR / per gate row block = 128
    # x packed [128, pipe, ic, pair, win, kind, 128]: per (p,ic,pair) the
    # (2 win x 2 kind x 128) block is contiguous -> 512B descriptors.
    x_p = nc.declare_dram_parameter("x", [128, NP, 4, npair, 2, 2, MV], FP8,
                                    isOutput=False)
    # DR1 stationary: 16*kh duplicated on both k-tiles (exact12) or
    # (16kh, kl) pair (round8).
    wkh_p = nc.declare_dram_parameter("wkh", [128, 4, 2, HID], FP8,
                                      isOutput=False)
    # DR2 stationary: kl, paired across ic via stride-2 slicing
    wkl_p = nc.declare_dram_parameter("wkl", [128, 4, HID], FP8,
                                      isOutput=False)
    # recurrent weights bf16(127*k_rh), kc-major blocks on partitions
    wrh_p = nc.declare_dram_parameter("wrh", [128, 4, HID], BF16,
                                      isOutput=False)
    # bias digit rows [128, 2, nck, 128]: rows 0..3 of tile0 = 16*digit
    bw_p = nc.declare_dram_parameter("bw", [128, 2, 4, 128], FP8,
                                     isOutput=False)
    # bias moving consts: rows 0..3 of tile0 = 128, 8, 0.5, 1/32
    bx_p = nc.declare_dram_parameter("bx", [128, 2, MV], FP8, isOutput=False)
    cf_p = nc.declare_dram_parameter("cf", [128, 1], F32, isOutput=False)
    # th output, all pipes merged: [p, t, nck, col]
    out_p = nc.declare_dram_parameter("out", [128, S, 4, N], BF16,
                                      isOutput=True)

    with tile.TileContext(nc) as tc, ExitStack() as ctx:
        const = ctx.enter_context(tc.tile_pool(name="const", bufs=1))
        # load order: bias operands first (bias matmuls lead window 0),
        # x-weights next (window-0 DR fills), recurrent weights (step 0),
        # scale last (first tanh is latest).
        bw_sb = const.tile([128, 2, 4, 128], FP8, tag="bw")
        nc.sync.dma_start(bw_sb[:], bw_p[:])
        bx_sb = const.tile([128, 2, MV], FP8, tag="bx")
        nc.sync.dma_start(bx_sb[:], bx_p[:])
        cf_sb = const.tile([128, 1], F32, tag="cf")
        nc.sync.dma_start(cf_sb[:], cf_p[:])
        wkh_sb = const.tile([128, 4, 2, HID], FP8, tag="wkh")
        nc.gpsimd.dma_start(wkh_sb[:], wkh_p[:])
        wkl_sb = const.tile([128, 4, HID], FP8, tag="wkl")
        nc.gpsimd.dma_start(wkl_sb[:], wkl_p[:])
        wrh_t = const.tile([128, 4, HID], BF16, tag="wrh")
        nc.gpsimd.dma_start(wrh_t[:], wrh_p[:])
        wrh_sb = [wrh_t[:, kc, :] for kc in range(4)]
        # Warm the ACT tanh table early.
        warm = const.tile([128, 1], F32, tag="warm")
        nc.scalar.activation(warm[:, 0:1], cf_sb[:, 0:1], ACTF.Tanh)

        pJ = ctx.enter_context(tc.tile_pool(name="pJ", bufs=4))
        pM = ctx.enter_context(tc.tile_pool(name="pM", bufs=3))
        psG = [ctx.enter_context(tc.tile_pool(name=f"psG{p}", bufs=2,
                                              space="PSUM")) for p in range(NP)]

        j_pairs = [None] * npair
        g_tiles = [[None] * nxb for _ in range(NP)]

        def dma_pair(pr):
            jt = pJ.tile([128, NP, 4, 2, 2, MV], FP8, name="j", tag="j")
            nc.sync.dma_start(jt[:], x_p[:, :, :, pr, :, :, :])
            j_pairs[pr] = jt

        jmm_queue = []  # deferred window-seeding matmuls, drained as PE filler

        def push_window(b):
            jt, w = j_pairs[b // 2], b % 2
            for p in range(NP):
                g = psG[p].tile([128, 4, MV], F32, name="g", tag="g")
                g_tiles[p][b] = g
                # bias DR seeds; nck0 leads the 2KB PSUM bank (start marks
                # the whole bank pending-zero; later writes to pending bytes
                # overwrite).
                for nck in range(4):
                    jmm_queue.append((
                        g[:, nck, :], bw_sb[:, :, nck, :], bx_sb[:],
                        nck == 0))
                # DR1: (j8, dj) x 16kh_ic  == j @ 16kh_ic exactly
                # (round8: (j8, j8) x (16kh_ic, kl_ic) == j8 @ k_ri_ic)
                for ic in range(4):
                    for nck in range(4):
                        jmm_queue.append((
                            g[:, nck, :],
                            wkh_sb[:, ic, :, nck * 128:(nck + 1) * 128],
                            jt[:, p, ic, w, :, :],
                            False))
                if J_MODE == "exact12":
                    # DR2: j8_A @ kl_A + j8_B @ kl_B, pairs (0,2),(1,3)
                    for ica in range(2):
                        for nck in range(4):
                            jmm_queue.append((
                                g[:, nck, :],
                                wkl_sb[:, ica:ica + 3:2,
                                       nck * 128:(nck + 1) * 128],
                                jt[:, p, ica:ica + 3:2, w, 0, :],
                                False))

        NFILL = NP * (4 + 16 + (8 if J_MODE == "exact12" else 0))

        def emit_jmm(n):
            for _ in range(n):
                if not jmm_queue:
                    return
                out, lhsT, rhs, start = jmm_queue.pop(0)
                nc.tensor.matmul(out, lhsT, rhs, start=start, stop=False,
                                 perf_mode=DR, skip_group_check=True)

        dma_pair(0)
        dma_pair(1)
        push_window(0)
        emit_jmm(len(jmm_queue))
        push_window(1)

        m0 = pM.tile([128, 1, 4, N], BF16, name="m0", tag="m0")
        nc.vector.memset(m0[:], 0.0)
        m_prev = [m0] * NP
        prev_slot = [0] * NP
        mslab = None

        for t in range(S):
            b, s = t // XB, t % XB
            sb_i, sb_st, sb_ln = slab_of[t]
            os = t - sb_st
            if s == 0 and b % 2 == 0 and 4 <= b + 4 < nxb:
                dma_pair((b + 4) // 2)
            if os == 0:
                mslab = pM.tile([128, sb_ln, 4, N], BF16, name="m", tag="m")
            for p in range(NP):
                gate = g_tiles[p][b]
                c0 = p * PC
                for kc in range(4):
                    for nck in range(4):
                        nc.tensor.matmul(
                            gate[:, nck, s * PC:(s + 1) * PC],
                            wrh_sb[kc][:, nck * 128:(nck + 1) * 128],
                            m_prev[p][:, prev_slot[p], kc, c0:c0 + PC],
                            start=False, stop=(kc == 3 and nck == 3),
                            skip_group_check=True,
                        )
                emit_jmm((NFILL // XB) // NP)
                nc.scalar.activation(mslab[:, os, :, c0:c0 + PC],
                                     gate[:, :, s * PC:(s + 1) * PC],
                                     ACTF.Tanh, scale=cf_sb[:, 0:1])
                m_prev[p], prev_slot[p] = mslab, os
            if s == XB - 1 and b + 2 < nxb:
                push_window(b + 2)
            if os == sb_ln - 1:
                eng = nc.scalar if sb_i == len(slabs) - 1 else nc.sync
                eng.dma_start(out_p[:, sb_st:sb_st + sb_ln, :, :], mslab[:])
    nc.compile()
    return nc


def _host_prep(inputs, seq):
    L = seq // NCHUNK
    S = L + WARM
    x = np.asarray(inputs["inputs"], np.float32)
    in_maps = []
    meta = []
    for d, (wri, wrh, b) in enumerate([
        (inputs["w_ri_f"], inputs["w_rh_f"], inputs["b_f"]),
        (inputs["w_ri_b"], inputs["w_rh_b"], inputs["b_b"]),
    ]):
        wri = np.asarray(wri, np.float32); wrh = np.asarray(wrh, np.float32)
        b = np.asarray(b, np.float32)
        threshold = np.float32(max(np.abs(wri).max(), np.abs(wrh).max()))
        s = np.float32(threshold / QMAX)
        k_ri = np.clip(np.round(wri / s), -QMAX, QMAX)
        k_rh = np.clip(np.round(wrh / s), -QMAX, QMAX)
        c_s = np.float32(np.float64(s) / 127.0)
        # x-weight split: k_ri = 16*kh + kl, kh/kl in [-8,8] (e4m3-exact)
        kh = np.round(k_ri / 16.0)
        kl = k_ri - 16.0 * kh
        assert np.abs(kh).max() <= 8 and np.abs(kl).max() <= 8
        # [128, ic, HID] layouts (partition = channel within ic block)
        wkh = (16.0 * kh).reshape(4, 128, HID).transpose(1, 0, 2)
        wklr = kl.reshape(4, 128, HID).transpose(1, 0, 2)
        if J_MODE == "exact12":
            wkhd = np.repeat(wkh[:, :, None, :], 2, axis=2)
        else:  # round8: k-tiles = (16kh, kl); moving = (j8, j8)
            wkhd = np.stack([wkh, wklr], axis=2)
        # bias digits: bias = 2048A + 128B + 8C + D/2, digits in [-8,8]
        bias_int = b.astype(np.float64) / np.float64(c_s)
        A = np.round(bias_int / 2048.0); r = bias_int - 2048.0 * A
        B = np.round(r / 128.0); r -= 128.0 * B
        C = np.round(r / 8.0); r -= 8.0 * C
        D = np.round(2.0 * r)
        assert max(np.abs(A).max(), np.abs(B).max(), np.abs(C).max(),
                   np.abs(D).max()) <= 8
        bw = np.zeros((128, 2, 4, 128), np.float64)
        for r_i, dig in enumerate((A, B, C, D)):
            bw[r_i, 0] = (16.0 * dig).reshape(4, 128)
        bx = np.zeros((128, 2, XB * PC), np.float32)
        for r_i, v in enumerate((128.0, 8.0, 0.5, 0.03125)):
            bx[r_i, 0, :] = v
        cf = np.full((128, 1), c_s, np.float32)
        meta.append((np.ascontiguousarray(wkhd.astype(E4)),
                     np.ascontiguousarray(wklr.astype(E4)),
                     np.ascontiguousarray(
                         ((127.0 * k_rh).reshape(4, 128, HID)
                          .transpose(1, 0, 2)).astype(ml_dtypes.bfloat16)),
                     np.ascontiguousarray(bw.astype(E4)),
                     np.ascontiguousarray(bx.astype(E4)), cf))
    xs = [x[:seq], x[:seq][::-1]]
    nxb = S // XB
    CPC = NCHUNK // 4  # chunks per core
    for core in range(8):
        d = core // 4
        wkhd, wklr, wrh_p, bw, bx, cf = meta[d]
        xd = xs[d]
        xT = np.empty((128, 4, S, NCOL), np.float32)
        for cl in range(CPC):
            q = CPC * (core % 4) + cl
            t0 = 0 if q == 0 else q * L - WARM
            blk = xd[t0:t0 + S]                     # [S, 16, 512]
            xT[:, :, :, cl * 16:(cl + 1) * 16] = (
                blk.transpose(2, 0, 1).reshape(4, 128, S, 16).transpose(1, 0, 2, 3))
        j = np.clip(np.round(127.0 * np.clip(xT, -1.0, 1.0)), -127.0, 127.0)
        j8 = j.astype(E4)
        dj = j - j8.astype(np.float32)
        assert np.abs(dj).max() <= 4
        if J_MODE != "exact12":
            dj = j8.astype(np.float32)  # second moving slot = j8 again

        # pack [128, pipe, ic, pair, win, kind, XB*PC]
        def pack(v):  # v [128, 4ic, S, NCOL]
            v = v.reshape(128, 4, nxb // 2, 2, XB, NP, PC)
            return v.transpose(0, 5, 1, 2, 3, 4, 6)  # [128,NP,4,pair,win,XB,PC]
        xp = np.stack([pack(j8.astype(np.float32)), pack(dj)], axis=5)
        xp = np.ascontiguousarray(
            xp.reshape(128, NP, 4, nxb // 2, 2, 2, XB * PC).astype(E4))
        in_maps.append({"x": xp, "wkh": wkhd, "wkl": wklr, "wrh": wrh_p,
                        "bw": bw, "bx": bx, "cf": cf})
    return in_maps


def _run(inputs, seq=SEQ, tb=None, trace=False):
    L = seq // NCHUNK
    S = L + WARM
    assert seq % NCHUNK == 0 and S % XB == 0 and (S // XB) % 2 == 0
    key = _cache_key(seq)
    if key not in _cache:
        _cache[key] = _build(S, NCOL)
    nc = _cache[key]
    in_maps = _host_prep(inputs, seq)
    res = run_bass_kernel_spmd(nc, in_maps, core_ids=list(range(8)), trace=trace)
    out = np.empty((seq, BATCH, 2 * HID), np.float32)
    CPC = NCHUNK // 4
    for core in range(8):
        d = core // 4
        th = np.asarray(res.results[core]["out"], dtype=np.float32)
        m = np.clip(np.round(127.0 * th), -127.0, 127.0)
        h = m / np.float32(127.0)
        h = h.transpose(1, 3, 2, 0).reshape(S, NCOL, HID)  # [S, n, hid]
        for cl in range(CPC):
            q = CPC * (core % 4) + cl
            lo = 0 if q == 0 else WARM
            sl = h[lo:lo + L, cl * 16:(cl + 1) * 16, :]    # [L, 16, 512]
            if d == 0:
                out[q * L:(q + 1) * L, :, :HID] = sl
            else:
                out[seq - (q + 1) * L:seq - q * L, :, HID:] = sl[::-1]
    return out, res


def kernel(**inputs):
    out, _ = _run(inputs)
    return out


# revision 19
# speedup vs baseline: 1.1883x; 1.0129x over previous
"""Bidirectional quantized RNN (fake-quant int8 weights/acts) on 8 trn2 cores.

Sequence-parallel sharding: each direction split into NCHUNK chunks of
L=seq/NCHUNK steps with W warmup steps (outputs discarded, chunk 0 exact);
core c handles direction c//4 and chunks (NCHUNK//4)*(c%4).. -> NCOL columns,
S=L+W steps.  The recurrent state is the bf16 tanh output th (validated:
bf16 rounding is washed out by the contraction).

Geometry tuned against the TimelineSim cost model (matmul cost =
out_free_rows * pe_cycle * cyc_per_row with fp8e4 DoubleRow = 0.5 cyc/row;
ACT op = free*0.833ns + ~185ns access + ~450ns round-trip latency to the
next dependent matmul):

* NCHUNK=32 -> 128 columns/core over 4 pipes x 32 cols, S=72 steps: the
  tanh->matmul round trip (~950ns) hides under ~1600ns/step of PE work,
  and each ACT op is big enough (free=128) to amortize its fixed cost.
* XB=4-step PSUM gate windows: 4 pipes x 2 windows x 1 bank = 8 banks.
* x-side matmuls in fp8e4 DoubleRow, EXACT at 0.75x bf16 row cost:
  j = j8 + dj (j8 = e4m3(j); dj = j - j8 is a small int <= 4, e4m3-exact);
  k_ri = 16*kh + kl (kh, kl in [-8,8], e4m3-exact).
    DR1(ic):      j8 @ 16kh_ic + dj @ 16kh_ic  (= j @ 16kh_ic exact)
    DR2(icA,icB): j8_A @ kl_A  + j8_B @ kl_B   (pairs two ic blocks)
  dropping only dj@kl (~0.002% of gate RMS).  J_MODE='round8' instead packs
  (j8@16kh + j8@kl) in ONE DR per ic (0.5x cost, ~1.5% gate noise).
* bias seeded by one fp8 DR matmul per (nck, window): bias_int = 2048A +
  128B + 8C + D/2 digit rows (e4m3-exact) against const rows 128,8,.5,1/32.
* recurrent matmuls stay EXACT bf16 (th moving, bf16(127*k_rh) stationary).
* x DMAs fetch two windows at a time (512B descriptors); one output DMA
  per 8-step slab.
"""
import numpy as np
import ml_dtypes
from contextlib import ExitStack

import concourse.bacc as bacc
import concourse.tile as tile
import concourse.mybir as mybir
from concourse.bass_utils import run_bass_kernel_spmd

SEQ, BATCH, IN, HID = 2048, 16, 512, 512
QMAX = np.float32(127.0)
F32 = mybir.dt.float32
BF16 = mybir.dt.bfloat16
FP8 = mybir.dt.float8e4
ACTF = mybir.ActivationFunctionType
DR = mybir.MatmulPerfMode.DoubleRow
E4 = ml_dtypes.float8_e4m3

NCHUNK = 32     # chunks per direction
WARM = 8        # cold-start warmup steps per chunk (chunk 0: exact anyway)
XB = 4          # steps per PSUM gate window / x block
NCOL = (NCHUNK // 4) * BATCH  # columns per core = 128
NP = 4          # pipes
PC = NCOL // NP               # columns per pipe = 32
J_MODE = "exact12"            # 'exact12' | 'round8'

_cache = {}


def _cache_key(seq):
    return (seq, 128 if seq >= 128 else 32, J_MODE, WARM, NCHUNK)


def _slab_schedule(S):
    if S % 8 == 0 and S >= 16:
        return [8] * (S // 8 - 1) + [4, 2, 2]
    for ob in (8, 17, 16, 34):
        if S % ob == 0:
            return [ob] * (S // ob)
    raise ValueError(S)


def _build(S, N):
    """One SPMD program for all 8 cores. S = L+W local steps, N columns."""
    assert N == NCOL
    slabs = _slab_schedule(S)
    slab_start = []
    t0 = 0
    for ln in slabs:
        slab_start.append(t0)
        t0 += ln
    slab_of = {}
    for i, (st, ln) in enumerate(zip(slab_start, slabs)):
        for t in range(st, st + ln):
            slab_of[t] = (i, st, ln)
    nc = bacc.Bacc("TRN2")
    nxb = S // XB
    npair = nxb // 2
    MV = XB * PC  # moving size per DR / per gate row block = 128
    # x packed [128, pipe, ic, pair, win, kind, 128]: per (p,ic,pair) the
    # (2 win x 2 kind x 128) block is contiguous -> 512B descriptors.
    x_p = nc.declare_dram_parameter("x", [128, NP, 4, npair, 2, 2, MV], FP8,
                                    isOutput=False)
    # DR1 stationary: 16*kh duplicated on both k-tiles (exact12) or
    # (16kh, kl) pair (round8).
    wkh_p = nc.declare_dram_parameter("wkh", [128, 4, 2, HID], FP8,
                                      isOutput=False)
    # DR2 stationary: kl, paired across ic via stride-2 slicing
    wkl_p = nc.declare_dram_parameter("wkl", [128, 4, HID], FP8,
                                      isOutput=False)
    # recurrent weights bf16(127*k_rh), kc-major blocks on partitions
    wrh_p = nc.declare_dram_parameter("wrh", [128, 4, HID], BF16,
                                      isOutput=False)
    # bias digit rows [..., :512] (nck-major 128 chunks): rows 0..3 of tile0
    # = 16*digit; bias moving consts [..., 512:512+MV]: rows = 128,8,.5,1/32
    bwx_p = nc.declare_dram_parameter("bwx", [128, 2, 512 + MV], FP8,
                                      isOutput=False)
    cf_p = nc.declare_dram_parameter("cf", [128, 1], F32, isOutput=False)
    # th output, all pipes merged: [p, t, nck, col]
    out_p = nc.declare_dram_parameter("out", [128, S, 4, N], BF16,
                                      isOutput=True)

    with tile.TileContext(nc) as tc, ExitStack() as ctx:
        const = ctx.enter_context(tc.tile_pool(name="const", bufs=1))
        pJ = ctx.enter_context(tc.tile_pool(name="pJ", bufs=4))
        pM = ctx.enter_context(tc.tile_pool(name="pM", bufs=3))
        psG = [ctx.enter_context(tc.tile_pool(name=f"psG{p}", bufs=2,
                                              space="PSUM")) for p in range(NP)]

        j_pairs = [None] * npair
        g_tiles = [[None] * nxb for _ in range(NP)]

        def dma_pair(pr, split=False):
            jt = pJ.tile([128, NP, 4, 2, 2, MV], FP8, name="j", tag="j")
            if split:  # pipe 0 lands first so window-0 seeding starts early
                nc.sync.dma_start(jt[:, 0, :, :, :, :], x_p[:, 0, :, pr, :, :, :])
                nc.sync.dma_start(jt[:, 1:, :, :, :, :], x_p[:, 1:, :, pr, :, :, :])
            else:
                nc.sync.dma_start(jt[:], x_p[:, :, :, pr, :, :, :])
            j_pairs[pr] = jt

        # prologue DMA order tuned for the serial HWDGE/DMA devices: bias
        # operands (lead window 0), x pipe-0 of pair 0, recurrent weights
        # (step 0), the rest of pair 0; x-weights ride the gpsimd SWDGE
        # queue in parallel.
        bwx_sb = const.tile([128, 2, 512 + MV], FP8, tag="bwx")
        nc.sync.dma_start(bwx_sb[:], bwx_p[:])
        bw_sb = bwx_sb[:, :, 0:512]
        bx_sb = bwx_sb[:, :, 512:512 + MV]
        wkh_sb = const.tile([128, 4, 2, HID], FP8, tag="wkh")
        nc.gpsimd.dma_start(wkh_sb[:], wkh_p[:])
        wkl_sb = const.tile([128, 4, HID], FP8, tag="wkl")
        nc.gpsimd.dma_start(wkl_sb[:], wkl_p[:])
        cf_sb = const.tile([128, 1], F32, tag="cf")
        nc.gpsimd.dma_start(cf_sb[:], cf_p[:])
        dma_pair(0, split=True)
        wrh_t = const.tile([128, 4, HID], BF16, tag="wrh")
        nc.sync.dma_start(wrh_t[:], wrh_p[:])
        wrh_sb = [wrh_t[:, kc, :] for kc in range(4)]
        # Warm the ACT tanh table early.
        warm = const.tile([128, 1], F32, tag="warm")
        nc.scalar.activation(warm[:, 0:1], cf_sb[:, 0:1], ACTF.Tanh)

        jmm_queue = []  # deferred window-seeding matmuls, drained as PE filler

        def push_window(b):
            jt, w = j_pairs[b // 2], b % 2
            for p in range(NP):
                g = psG[p].tile([128, 4, MV], F32, name="g", tag="g")
                g_tiles[p][b] = g
                # bias DR seeds; nck0 leads the 2KB PSUM bank (start marks
                # the whole bank pending-zero; later writes to pending bytes
                # overwrite).
                for nck in range(4):
                    jmm_queue.append((
                        g[:, nck, :], bw_sb[:, :, nck * 128:(nck + 1) * 128],
                        bx_sb[:], nck == 0))
                # DR1: (j8, dj) x 16kh_ic  == j @ 16kh_ic exactly
                # (round8: (j8, j8) x (16kh_ic, kl_ic) == j8 @ k_ri_ic)
                for ic in range(4):
                    for nck in range(4):
                        jmm_queue.append((
                            g[:, nck, :],
                            wkh_sb[:, ic, :, nck * 128:(nck + 1) * 128],
                            jt[:, p, ic, w, :, :],
                            False))
                if J_MODE == "exact12":
                    # DR2: j8_A @ kl_A + j8_B @ kl_B, pairs (0,2),(1,3)
                    for ica in range(2):
                        for nck in range(4):
                            jmm_queue.append((
                                g[:, nck, :],
                                wkl_sb[:, ica:ica + 3:2,
                                       nck * 128:(nck + 1) * 128],
                                jt[:, p, ica:ica + 3:2, w, 0, :],
                                False))

        NFILL = NP * (4 + 16 + (8 if J_MODE == "exact12" else 0))

        def emit_jmm(n):
            for _ in range(n):
                if not jmm_queue:
                    return
                out, lhsT, rhs, start = jmm_queue.pop(0)
                nc.tensor.matmul(out, lhsT, rhs, start=start, stop=False,
                                 perf_mode=DR, skip_group_check=True)

        dma_pair(1)
        push_window(0)
        emit_jmm(len(jmm_queue))
        push_window(1)

        m0 = pM.tile([128, 1, 4, N], BF16, name="m0", tag="m0")
        nc.vector.memset(m0[:], 0.0)
        m_prev = [m0] * NP
        prev_slot = [0] * NP
        mslab = None

        for t in range(S):
            b, s = t // XB, t % XB
            sb_i, sb_st, sb_ln = slab_of[t]
            os = t - sb_st
            if s == 0 and b % 2 == 0 and 4 <= b + 4 < nxb:
                dma_pair((b + 4) // 2)
            if os == 0:
                mslab = pM.tile([128, sb_ln, 4, N], BF16, name="m", tag="m")
            for p in range(NP):
                gate = g_tiles[p][b]
                c0 = p * PC
                for kc in range(4):
                    for nck in range(4):
                        nc.tensor.matmul(
                            gate[:, nck, s * PC:(s + 1) * PC],
                            wrh_sb[kc][:, nck * 128:(nck + 1) * 128],
                            m_prev[p][:, prev_slot[p], kc, c0:c0 + PC],
                            start=False, stop=(kc == 3 and nck == 3),
                            skip_group_check=True,
                        )
                emit_jmm((NFILL // XB) // NP)
                nc.scalar.activation(mslab[:, os, :, c0:c0 + PC],
                                     gate[:, :, s * PC:(s + 1) * PC],
                                     ACTF.Tanh, scale=cf_sb[:, 0:1])
                m_prev[p], prev_slot[p] = mslab, os
            if s == XB - 1 and b + 2 < nxb:
                push_window(b + 2)
            if os == sb_ln - 1:
                eng = nc.scalar if sb_i == len(slabs) - 1 else nc.sync
                eng.dma_start(out_p[:, sb_st:sb_st + sb_ln, :, :], mslab[:])
    nc.compile()
    return nc


def _host_prep(inputs, seq):
    L = seq // NCHUNK
    S = L + WARM
    x = np.asarray(inputs["inputs"], np.float32)
    in_maps = []
    meta = []
    for d, (wri, wrh, b) in enumerate([
        (inputs["w_ri_f"], inputs["w_rh_f"], inputs["b_f"]),
        (inputs["w_ri_b"], inputs["w_rh_b"], inputs["b_b"]),
    ]):
        wri = np.asarray(wri, np.float32); wrh = np.asarray(wrh, np.float32)
        b = np.asarray(b, np.float32)
        threshold = np.float32(max(np.abs(wri).max(), np.abs(wrh).max()))
        s = np.float32(threshold / QMAX)
        k_ri = np.clip(np.round(wri / s), -QMAX, QMAX)
        k_rh = np.clip(np.round(wrh / s), -QMAX, QMAX)
        c_s = np.float32(np.float64(s) / 127.0)
        # x-weight split: k_ri = 16*kh + kl, kh/kl in [-8,8] (e4m3-exact)
        kh = np.round(k_ri / 16.0)
        kl = k_ri - 16.0 * kh
        assert np.abs(kh).max() <= 8 and np.abs(kl).max() <= 8
        # [128, ic, HID] layouts (partition = channel within ic block)
        wkh = (16.0 * kh).reshape(4, 128, HID).transpose(1, 0, 2)
        wklr = kl.reshape(4, 128, HID).transpose(1, 0, 2)
        if J_MODE == "exact12":
            wkhd = np.repeat(wkh[:, :, None, :], 2, axis=2)
        else:  # round8: k-tiles = (16kh, kl); moving = (j8, j8)
            wkhd = np.stack([wkh, wklr], axis=2)
        # bias digits: bias = 2048A + 128B + 8C + D/2, digits in [-8,8]
        bias_int = b.astype(np.float64) / np.float64(c_s)
        A = np.round(bias_int / 2048.0); r = bias_int - 2048.0 * A
        B = np.round(r / 128.0); r -= 128.0 * B
        C = np.round(r / 8.0); r -= 8.0 * C
        D = np.round(2.0 * r)
        assert max(np.abs(A).max(), np.abs(B).max(), np.abs(C).max(),
                   np.abs(D).max()) <= 8
        bwx = np.zeros((128, 2, 512 + XB * PC), np.float64)
        for r_i, dig in enumerate((A, B, C, D)):
            bwx[r_i, 0, 0:512] = (16.0 * dig)
        for r_i, v in enumerate((128.0, 8.0, 0.5, 0.03125)):
            bwx[r_i, 0, 512:] = v
        cf = np.full((128, 1), c_s, np.float32)
        meta.append((np.ascontiguousarray(wkhd.astype(E4)),
                     np.ascontiguousarray(wklr.astype(E4)),
                     np.ascontiguousarray(
                         ((127.0 * k_rh).reshape(4, 128, HID)
                          .transpose(1, 0, 2)).astype(ml_dtypes.bfloat16)),
                     np.ascontiguousarray(bwx.astype(E4)), cf))
    xs = [x[:seq], x[:seq][::-1]]
    nxb = S // XB
    CPC = NCHUNK // 4  # chunks per core
    for core in range(8):
        d = core // 4
        wkhd, wklr, wrh_p, bwx, cf = meta[d]
        xd = xs[d]
        xT = np.empty((128, 4, S, NCOL), np.float32)
        for cl in range(CPC):
            q = CPC * (core % 4) + cl
            t0 = 0 if q == 0 else q * L - WARM
            blk = xd[t0:t0 + S]                     # [S, 16, 512]
            xT[:, :, :, cl * 16:(cl + 1) * 16] = (
                blk.transpose(2, 0, 1).reshape(4, 128, S, 16).transpose(1, 0, 2, 3))
        j = np.clip(np.round(127.0 * np.clip(xT, -1.0, 1.0)), -127.0, 127.0)
        j8 = j.astype(E4)
        dj = j - j8.astype(np.float32)
        assert np.abs(dj).max() <= 4
        if J_MODE != "exact12":
            dj = j8.astype(np.float32)  # second moving slot = j8 again

        # pack [128, pipe, ic, pair, win, kind, XB*PC]
        def pack(v):  # v [128, 4ic, S, NCOL]
            v = v.reshape(128, 4, nxb // 2, 2, XB, NP, PC)
            return v.transpose(0, 5, 1, 2, 3, 4, 6)  # [128,NP,4,pair,win,XB,PC]
        xp = np.stack([pack(j8.astype(np.float32)), pack(dj)], axis=5)
        xp = np.ascontiguousarray(
            xp.reshape(128, NP, 4, nxb // 2, 2, 2, XB * PC).astype(E4))
        in_maps.append({"x": xp, "wkh": wkhd, "wkl": wklr, "wrh": wrh_p,
                        "bwx": bwx, "cf": cf})
    return in_maps


def _run(inputs, seq=SEQ, tb=None, trace=False):
    L = seq // NCHUNK
    S = L + WARM
    assert seq % NCHUNK == 0 and S % XB == 0 and (S // XB) % 2 == 0
    key = _cache_key(seq)
    if key not in _cache:
        _cache[key] = _build(S, NCOL)
    nc = _cache[key]
    in_maps = _host_prep(inputs, seq)
    res = run_bass_kernel_spmd(nc, in_maps, core_ids=list(range(8)), trace=trace)
    out = np.empty((seq, BATCH, 2 * HID), np.float32)
    CPC = NCHUNK // 4
    for core in range(8):
        d = core // 4
        th = np.asarray(res.results[core]["out"], dtype=np.float32)
        m = np.clip(np.round(127.0 * th), -127.0, 127.0)
        h = m / np.float32(127.0)
        h = h.transpose(1, 3, 2, 0).reshape(S, NCOL, HID)  # [S, n, hid]
        for cl in range(CPC):
            q = CPC * (core % 4) + cl
            lo = 0 if q == 0 else WARM
            sl = h[lo:lo + L, cl * 16:(cl + 1) * 16, :]    # [L, 16, 512]
            if d == 0:
                out[q * L:(q + 1) * L, :, :HID] = sl
            else:
                out[seq - (q + 1) * L:seq - q * L, :, HID:] = sl[::-1]
    return out, res


def kernel(**inputs):
    out, _ = _run(inputs)
    return out


# revision 26
# speedup vs baseline: 1.4053x; 1.1826x over previous
"""Bidirectional quantized RNN (fake-quant int8 weights/acts) on 8 trn2 cores.

Sequence-parallel sharding: each direction split into NCHUNK chunks of
L=seq/NCHUNK steps with W warmup steps (outputs discarded, chunk 0 exact);
core c handles direction c//4 and chunks (NCHUNK//4)*(c%4).. -> NCOL columns,
S=L+W steps.  The recurrent state is the bf16 tanh output th (validated:
bf16 rounding is washed out by the contraction).

Geometry tuned against the TimelineSim cost model (matmul cost =
out_free_rows * pe_cycle * cyc_per_row with fp8e4 DoubleRow = 0.5 cyc/row;
ACT op = free*0.833ns + ~185ns access + ~450ns round-trip latency to the
next dependent matmul):

* NCHUNK=32 -> 128 columns/core over 4 pipes x 32 cols, S=72 steps: the
  tanh->matmul round trip (~950ns) hides under ~1600ns/step of PE work,
  and each ACT op is big enough (free=128) to amortize its fixed cost.
* XB=4-step PSUM gate windows: 4 pipes x 2 windows x 1 bank = 8 banks.
* x-side matmuls in fp8e4 DoubleRow, EXACT at 0.75x bf16 row cost:
  j = j8 + dj (j8 = e4m3(j); dj = j - j8 is a small int <= 4, e4m3-exact);
  k_ri = 16*kh + kl (kh, kl in [-8,8], e4m3-exact).
    DR1(ic):      j8 @ 16kh_ic + dj @ 16kh_ic  (= j @ 16kh_ic exact)
    DR2(icA,icB): j8_A @ kl_A  + j8_B @ kl_B   (pairs two ic blocks)
  dropping only dj@kl (~0.002% of gate RMS).  J_MODE='round8' instead packs
  (j8@16kh + j8@kl) in ONE DR per ic (0.5x cost, ~1.5% gate noise).
* bias seeded by one fp8 DR matmul per (nck, window): bias_int = 2048A +
  128B + 8C + D/2 digit rows (e4m3-exact) against const rows 128,8,.5,1/32.
* recurrent matmuls stay EXACT bf16 (th moving, bf16(127*k_rh) stationary).
* x DMAs fetch two windows at a time (512B descriptors); one output DMA
  per 8-step slab.
"""
import numpy as np
import ml_dtypes
from contextlib import ExitStack

import concourse.bacc as bacc
import concourse.tile as tile
import concourse.mybir as mybir
from concourse.bass_utils import run_bass_kernel_spmd

SEQ, BATCH, IN, HID = 2048, 16, 512, 512
QMAX = np.float32(127.0)
F32 = mybir.dt.float32
BF16 = mybir.dt.bfloat16
FP8 = mybir.dt.float8e4
ACTF = mybir.ActivationFunctionType
DR = mybir.MatmulPerfMode.DoubleRow
E4 = ml_dtypes.float8_e4m3

import os
NCHUNK = 32     # chunks per direction
WARM = int(os.environ.get("K_WARM", "8"))   # cold-start warmup steps/chunk
XB = 4          # steps per PSUM gate window / x block
NCOL = (NCHUNK // 4) * BATCH  # columns per core = 128
NP = 4          # pipes
PC = NCOL // NP               # columns per pipe = 32
J_MODE = os.environ.get("K_JMODE", "exact12")  # 'exact12' | 'round8'

_cache = {}


def _cache_key(seq):
    return (seq, 128 if seq >= 128 else 32, J_MODE, WARM, NCHUNK)


def _slab_schedule(S):
    assert S % 4 == 0 and S >= 16
    out = [8] * (S // 8)
    if S % 8:
        out.append(4)
    tail = out.pop()  # split the tail for a short final dependency chain
    out.extend([4, 2, 1, 1] if tail == 8 else [2, 1, 1])
    return out


def _build(S, N):
    """One SPMD program for all 8 cores. S = L+W local steps, N columns."""
    assert N == NCOL
    slabs = _slab_schedule(S)
    slab_start = []
    t0 = 0
    for ln in slabs:
        slab_start.append(t0)
        t0 += ln
    slab_of = {}
    for i, (st, ln) in enumerate(zip(slab_start, slabs)):
        for t in range(st, st + ln):
            slab_of[t] = (i, st, ln)
    nc = bacc.Bacc("TRN2")
    nxb = S // XB
    npair = (nxb + 1) // 2  # odd nxb: final pair is half-padded
    MV = XB * PC  # moving size per DR / per gate row block = 128
    # x packed [128, pipe, ic, pair, win, kind, 128]: per (p,ic,pair) the
    # (2 win x 2 kind x 128) block is contiguous -> 512B descriptors.
    x_p = nc.declare_dram_parameter("x", [128, NP, 4, npair, 2, 2, MV], FP8,
                                    isOutput=False)
    # DR1 stationary: 16*kh duplicated on both k-tiles (exact12) or
    # (16kh, kl) pair (round8).
    wkh_p = nc.declare_dram_parameter("wkh", [128, 4, 2, HID], FP8,
                                      isOutput=False)
    # DR2 stationary: kl, paired across ic via stride-2 slicing
    wkl_p = nc.declare_dram_parameter("wkl", [128, 4, HID], FP8,
                                      isOutput=False)
    # recurrent weights bf16(127*k_rh), kc-major blocks on partitions
    wrh_p = nc.declare_dram_parameter("wrh", [128, 4, HID], BF16,
                                      isOutput=False)
    # bias digit rows [..., :512] (nck-major 128 chunks): rows 0..3 of tile0
    # = 16*digit; bias moving consts [..., 512:512+MV]: rows = 128,8,.5,1/32
    bwx_p = nc.declare_dram_parameter("bwx", [128, 2, 512 + MV], FP8,
                                      isOutput=False)
    cf_p = nc.declare_dram_parameter("cf", [128, 1], F32, isOutput=False)
    # th output, all pipes merged: [p, t, nck, col]
    out_p = nc.declare_dram_parameter("out", [128, S, 4, N], BF16,
                                      isOutput=True)

    with tile.TileContext(nc) as tc, ExitStack() as ctx:
        const = ctx.enter_context(tc.tile_pool(name="const", bufs=1))
        pJ = ctx.enter_context(tc.tile_pool(name="pJ", bufs=4))
        pM = ctx.enter_context(tc.tile_pool(name="pM", bufs=3))
        psG = [ctx.enter_context(tc.tile_pool(name=f"psG{p}", bufs=2,
                                              space="PSUM")) for p in range(NP)]

        j_pairs = [None] * npair
        g_tiles = [[None] * nxb for _ in range(NP)]

        def dma_pair(pr, split=False):
            jt = pJ.tile([128, NP, 4, 2, 2, MV], FP8, name="j", tag="j")
            if split:  # pipe 0 lands first so window-0 seeding starts early
                nc.sync.dma_start(jt[:, 0, :, :, :, :], x_p[:, 0, :, pr, :, :, :])
                nc.sync.dma_start(jt[:, 1:, :, :, :, :], x_p[:, 1:, :, pr, :, :, :])
            else:
                nc.sync.dma_start(jt[:], x_p[:, :, :, pr, :, :, :])
            j_pairs[pr] = jt

        # prologue DMA order tuned for the serial HWDGE/DMA devices: bias
        # operands (lead window 0), x pipe-0 of pair 0, recurrent weights
        # (step 0), the rest of pair 0; x-weights ride the gpsimd SWDGE
        # queue in parallel.
        bwx_sb = const.tile([128, 2, 512 + MV], FP8, tag="bwx")
        nc.sync.dma_start(bwx_sb[:], bwx_p[:])
        bw_sb = bwx_sb[:, :, 0:512]
        bx_sb = bwx_sb[:, :, 512:512 + MV]
        wkh_sb = const.tile([128, 4, 2, HID], FP8, tag="wkh")
        nc.gpsimd.dma_start(wkh_sb[:], wkh_p[:])
        wkl_sb = const.tile([128, 4, HID], FP8, tag="wkl")
        nc.gpsimd.dma_start(wkl_sb[:], wkl_p[:])
        cf_sb = const.tile([128, 1], F32, tag="cf")
        nc.gpsimd.dma_start(cf_sb[:], cf_p[:])
        dma_pair(0, split=True)
        wrh_t = const.tile([128, 4, HID], BF16, tag="wrh")
        nc.sync.dma_start(wrh_t[:], wrh_p[:])
        wrh_sb = [wrh_t[:, kc, :] for kc in range(4)]
        # Warm the ACT tanh table early.
        warm = const.tile([128, 1], F32, tag="warm")
        nc.scalar.activation(warm[:, 0:1], cf_sb[:, 0:1], ACTF.Tanh)

        jmm_queue = []  # deferred window-seeding matmuls, drained as PE filler

        def push_window(b):
            jt, w = j_pairs[b // 2], b % 2
            for p in range(NP):
                g = psG[p].tile([128, 4, MV], F32, name="g", tag="g")
                g_tiles[p][b] = g
                # bias DR seeds; nck0 leads the 2KB PSUM bank (start marks
                # the whole bank pending-zero; later writes to pending bytes
                # overwrite).
                for nck in range(4):
                    jmm_queue.append((
                        g[:, nck, :], bw_sb[:, :, nck * 128:(nck + 1) * 128],
                        bx_sb[:], nck == 0))
                # DR1: (j8, dj) x 16kh_ic  == j @ 16kh_ic exactly
                # (round8: (j8, j8) x (16kh_ic, kl_ic) == j8 @ k_ri_ic)
                for ic in range(4):
                    for nck in range(4):
                        jmm_queue.append((
                            g[:, nck, :],
                            wkh_sb[:, ic, :, nck * 128:(nck + 1) * 128],
                            jt[:, p, ic, w, :, :],
                            False))
                if J_MODE == "exact12":
                    # DR2: j8_A @ kl_A + j8_B @ kl_B, pairs (0,2),(1,3)
                    for ica in range(2):
                        for nck in range(4):
                            jmm_queue.append((
                                g[:, nck, :],
                                wkl_sb[:, ica:ica + 3:2,
                                       nck * 128:(nck + 1) * 128],
                                jt[:, p, ica:ica + 3:2, w, 0, :],
                                False))

        NFILL = NP * (4 + 16 + (8 if J_MODE == "exact12" else 0))

        def emit_jmm(n):
            for _ in range(n):
                if not jmm_queue:
                    return
                out, lhsT, rhs, start = jmm_queue.pop(0)
                nc.tensor.matmul(out, lhsT, rhs, start=start, stop=False,
                                 perf_mode=DR, skip_group_check=True)

        dma_pair(1)
        push_window(0)
        emit_jmm(len(jmm_queue))
        push_window(1)

        m0 = pM.tile([128, 1, 4, N], BF16, name="m0", tag="m0")
        nc.vector.memset(m0[:], 0.0)
        m_prev = [m0] * NP
        prev_slot = [0] * NP
        mslab = None

        for t in range(S):
            b, s = t // XB, t % XB
            sb_i, sb_st, sb_ln = slab_of[t]
            os = t - sb_st
            if s == 0 and b % 2 == 0 and 4 <= b + 4 < nxb:
                dma_pair((b + 4) // 2)
            if os == 0:
                mslab = pM.tile([128, sb_ln, 4, N], BF16, name="m", tag="m")
            for p in range(NP):
                gate = g_tiles[p][b]
                c0 = p * PC
                for kc in range(4):
                    for nck in range(4):
                        nc.tensor.matmul(
                            gate[:, nck, s * PC:(s + 1) * PC],
                            wrh_sb[kc][:, nck * 128:(nck + 1) * 128],
                            m_prev[p][:, prev_slot[p], kc, c0:c0 + PC],
                            start=False, stop=(kc == 3 and nck == 3),
                            skip_group_check=True,
                        )
                emit_jmm((NFILL // XB) // NP)
                nc.scalar.activation(mslab[:, os, :, c0:c0 + PC],
                                     gate[:, :, s * PC:(s + 1) * PC],
                                     ACTF.Tanh, scale=cf_sb[:, 0:1])
                m_prev[p], prev_slot[p] = mslab, os
            if s == XB - 1 and b + 2 < nxb:
                push_window(b + 2)
            if os == sb_ln - 1:
                eng = nc.scalar if sb_i == len(slabs) - 1 else nc.sync
                eng.dma_start(out_p[:, sb_st:sb_st + sb_ln, :, :], mslab[:])
    nc.compile()
    return nc


def _host_prep(inputs, seq):
    L = seq // NCHUNK
    S = L + WARM
    x = np.asarray(inputs["inputs"], np.float32)
    in_maps = []
    meta = []
    for d, (wri, wrh, b) in enumerate([
        (inputs["w_ri_f"], inputs["w_rh_f"], inputs["b_f"]),
        (inputs["w_ri_b"], inputs["w_rh_b"], inputs["b_b"]),
    ]):
        wri = np.asarray(wri, np.float32); wrh = np.asarray(wrh, np.float32)
        b = np.asarray(b, np.float32)
        threshold = np.float32(max(np.abs(wri).max(), np.abs(wrh).max()))
        s = np.float32(threshold / QMAX)
        k_ri = np.clip(np.round(wri / s), -QMAX, QMAX)
        k_rh = np.clip(np.round(wrh / s), -QMAX, QMAX)
        c_s = np.float32(np.float64(s) / 127.0)
        # x-weight split: k_ri = 16*kh + kl, kh/kl in [-8,8] (e4m3-exact)
        kh = np.round(k_ri / 16.0)
        kl = k_ri - 16.0 * kh
        assert np.abs(kh).max() <= 8 and np.abs(kl).max() <= 8
        # [128, ic, HID] layouts (partition = channel within ic block)
        wkh = (16.0 * kh).reshape(4, 128, HID).transpose(1, 0, 2)
        wklr = kl.reshape(4, 128, HID).transpose(1, 0, 2)
        if J_MODE == "exact12":
            wkhd = np.repeat(wkh[:, :, None, :], 2, axis=2)
        else:  # round8: k-tiles = (16kh, kl); moving = (j8, j8)
            wkhd = np.stack([wkh, wklr], axis=2)
        # bias digits: bias = 2048A + 128B + 8C + D/2, digits in [-8,8]
        bias_int = b.astype(np.float64) / np.float64(c_s)
        A = np.round(bias_int / 2048.0); r = bias_int - 2048.0 * A
        B = np.round(r / 128.0); r -= 128.0 * B
        C = np.round(r / 8.0); r -= 8.0 * C
        D = np.round(2.0 * r)
        assert max(np.abs(A).max(), np.abs(B).max(), np.abs(C).max(),
                   np.abs(D).max()) <= 8
        bwx = np.zeros((128, 2, 512 + XB * PC), np.float64)
        for r_i, dig in enumerate((A, B, C, D)):
            bwx[r_i, 0, 0:512] = (16.0 * dig)
        for r_i, v in enumerate((128.0, 8.0, 0.5, 0.03125)):
            bwx[r_i, 0, 512:] = v
        cf = np.full((128, 1), c_s, np.float32)
        meta.append((np.ascontiguousarray(wkhd.astype(E4)),
                     np.ascontiguousarray(wklr.astype(E4)),
                     np.ascontiguousarray(
                         ((127.0 * k_rh).reshape(4, 128, HID)
                          .transpose(1, 0, 2)).astype(ml_dtypes.bfloat16)),
                     np.ascontiguousarray(bwx.astype(E4)), cf))
    xs = [x[:seq], x[:seq][::-1]]
    nxb = S // XB
    npair = (nxb + 1) // 2
    S_pad = npair * 2 * XB
    CPC = NCHUNK // 4  # chunks per core
    for core in range(8):
        d = core // 4
        wkhd, wklr, wrh_p, bwx, cf = meta[d]
        xd = xs[d]
        xT = np.empty((128, 4, S, NCOL), np.float32)
        for cl in range(CPC):
            q = CPC * (core % 4) + cl
            t0 = 0 if q == 0 else q * L - WARM
            blk = xd[t0:t0 + S]                     # [S, 16, 512]
            xT[:, :, :, cl * 16:(cl + 1) * 16] = (
                blk.transpose(2, 0, 1).reshape(4, 128, S, 16).transpose(1, 0, 2, 3))
        j = np.clip(np.round(127.0 * np.clip(xT, -1.0, 1.0)), -127.0, 127.0)
        j8 = j.astype(E4)
        dj = j - j8.astype(np.float32)
        assert np.abs(dj).max() <= 4
        if J_MODE != "exact12":
            dj = j8.astype(np.float32)  # second moving slot = j8 again

        if S_pad != S:
            pad = np.zeros((128, 4, S_pad - S, NCOL), np.float32)
            j8 = np.concatenate([j8.astype(np.float32), pad], axis=2).astype(E4)
            dj = np.concatenate([dj, pad], axis=2)

        # pack [128, pipe, ic, pair, win, kind, XB*PC]
        def pack(v):  # v [128, 4ic, S_pad, NCOL]
            v = v.reshape(128, 4, npair, 2, XB, NP, PC)
            return v.transpose(0, 5, 1, 2, 3, 4, 6)  # [128,NP,4,pair,win,XB,PC]
        xp = np.stack([pack(j8.astype(np.float32)), pack(dj)], axis=5)
        xp = np.ascontiguousarray(
            xp.reshape(128, NP, 4, npair, 2, 2, XB * PC).astype(E4))
        in_maps.append({"x": xp, "wkh": wkhd, "wkl": wklr, "wrh": wrh_p,
                        "bwx": bwx, "cf": cf})
    return in_maps


def _run(inputs, seq=SEQ, tb=None, trace=False):
    L = seq // NCHUNK
    S = L + WARM
    assert seq % NCHUNK == 0 and S % XB == 0
    key = _cache_key(seq)
    if key not in _cache:
        _cache[key] = _build(S, NCOL)
    nc = _cache[key]
    in_maps = _host_prep(inputs, seq)
    res = run_bass_kernel_spmd(nc, in_maps, core_ids=list(range(8)), trace=trace)
    out = np.empty((seq, BATCH, 2 * HID), np.float32)
    CPC = NCHUNK // 4
    for core in range(8):
        d = core // 4
        th = np.asarray(res.results[core]["out"], dtype=np.float32)
        m = np.clip(np.round(127.0 * th), -127.0, 127.0)
        h = m / np.float32(127.0)
        h = h.transpose(1, 3, 2, 0).reshape(S, NCOL, HID)  # [S, n, hid]
        for cl in range(CPC):
            q = CPC * (core % 4) + cl
            lo = 0 if q == 0 else WARM
            sl = h[lo:lo + L, cl * 16:(cl + 1) * 16, :]    # [L, 16, 512]
            if d == 0:
                out[q * L:(q + 1) * L, :, :HID] = sl
            else:
                out[seq - (q + 1) * L:seq - q * L, :, HID:] = sl[::-1]
    return out, res


def kernel(**inputs):
    out, _ = _run(inputs)
    return out
